# revision 38
# baseline (speedup 1.0000x reference)
"""Fused attention kernel for Trainium2, SPMD over 8 NeuronCores.

Problem: nn_Attention_2808908611625
  q = primary @ Wq + bq;  k = ctx @ Wk + bk;  v = ctx @ Wv + bv
  out = softmax(q k^T / sqrt(1024) - 1e9 * mask) @ v

Sharding: core c handles batch b = c//2, query-row half h = c%2
  (1024 query rows per core, full K/V context of its batch, K/V projection
  split across the core pair and exchanged with a pair AllGather).

Per-core pipeline:
  1. SWDGE cast-DMA fp32->bf16 loads (W first: K-proj critical path), PE
     128x128 transposes put the contraction dim on SBUF partitions.
  2. Q/K/V projections on PE (bf16, fp32 PSUM); q/k evicted as fp8 e4m3
     (bias folded); K exchanged through DRAM in fp8; bv added at the end
     (softmax rows sum to 1 => attn @ (1 bv^T) = bv).
  3. S = qT.T @ kT with fp8 DoubleRow matmuls ([128 x 512] PSUM tiles);
     mask folded with one DVE scalar_tensor_tensor (S += -960 * mask);
     P = exp(S/32) via ACT with accum_out row-sums. No max-subtraction:
     |S/32| <= ~4 unmasked, masked entries become exp(-30) ~ 1e-13.
  4. P tiles transposed by XBAR DMA (SBUF->SBUF, one per query tile),
     PV matmul in bf16, evict with per-partition 1/rowsum scale, add
     broadcast bv, one fp32 DMA out per query tile.
"""

import numpy as np

import concourse.bass as bass
import concourse.mybir as mybir
import concourse.tile as tile
from concourse import bacc, bass_utils
from concourse.masks import make_identity

BF = mybir.dt.bfloat16
F8 = mybir.dt.float8e4
F32 = mybir.dt.float32
AF = mybir.ActivationFunctionType
ALU = mybir.AluOpType
AX = mybir.AxisListType
DR = mybir.MatmulPerfMode.DoubleRow

B, LQ, LKV, D = 4, 2048, 2048, 1024
P = 128
LQ_LOC = (B * LQ) // 8  # 1024 query rows per core
DC = D // P             # 8 contraction chunks
M = D // P              # 8 output-dim chunks
QT = LQ_LOC // P        # 8 query tiles per core
NT = 512                # moving free dim / psum tile width
LT = LKV // NT          # 4 kv column tiles for S
LC = LKV // P           # 16 kv chunks for PV
HKV = LKV // 2          # per-core K/V rows (pair-sharded)
LTH = HKV // NT         # 2 own kv column tiles
LCH = HKV // P          # 8 own kv chunks
NB = LQ_LOC // P        # 8 row blocks per input tensor

UNROLL_REPS = False
PHASE1_ONLY = False  # timing probe: stop after projections+exchange


def build_nc(reps: int = 1):
    nc = bacc.Bacc("TRN2", num_swdge_queues=4, num_devices=8)

    x_d = nc.dram_tensor("primary", (LQ_LOC, D), F32, kind="ExternalInput")
    ctx_d = nc.dram_tensor("context_sequence", (LKV // 2, D), F32, kind="ExternalInput")
    mask_d = nc.dram_tensor("mask", (LQ_LOC, LKV), F32, kind="ExternalInput")
    wq_d = nc.dram_tensor("Wq", (D, D), F32, kind="ExternalInput")
    bq_d = nc.dram_tensor("bq", (D,), F32, kind="ExternalInput")
    wk_d = nc.dram_tensor("Wk", (D, D), F32, kind="ExternalInput")
    bk_d = nc.dram_tensor("bk", (D,), F32, kind="ExternalInput")
    wv_d = nc.dram_tensor("Wv", (D, D), F32, kind="ExternalInput")
    bv_d = nc.dram_tensor("bv", (D,), F32, kind="ExternalInput")
    out_d = nc.dram_tensor("out", (LQ_LOC, D), F32, kind="ExternalOutput")

    with tile.TileContext(nc) as tc:
        with (
            tc.tile_pool(name="const", bufs=1) as const,
            tc.tile_pool(name="persist", bufs=1) as persist,
            tc.tile_pool(name="mpool", bufs=1) as mpool,
            tc.tile_pool(name="dram", bufs=1, space="DRAM") as dram,
            tc.tile_pool(name="mmps", bufs=4, space="PSUM") as mmps,
            tc.tile_pool(name="tps", bufs=2, space="PSUM") as tps,
            tc.tile_pool(name="avps", bufs=1, space="PSUM") as avps,
        ):
            ident = const.tile([P, P], BF)
            make_identity(nc, ident)

            # biases: b*_sb[p, m] = b[m*128 + p]
            bq_sb = const.tile([P, M], F32)
            bk_sb = const.tile([P, M], F32)
            with nc.allow_non_contiguous_dma(reason="tiny bias vectors"):
                nc.sync.dma_start(bq_sb, bq_d[:].rearrange("(m p) -> p m", p=P))
                nc.sync.dma_start(bk_sb, bk_d[:].rearrange("(m p) -> p m", p=P))

            # bv broadcast to all partitions: ones[1,128].T @ bv[1, D]
            bv_row = const.tile([1, D], BF)
            nc.gpsimd.dma_start(bv_row, bv_d[:].rearrange("(one n) -> one n", one=1))
            ones_row = const.tile([1, P], BF)
            nc.vector.memset(ones_row, 1.0)
            bv_bcast = const.tile([P, D], F32)

            qT = persist.tile([P, M, LQ_LOC], F8)   # q^T   [dattn, lq] (e4m3)
            kT = persist.tile([P, M, LKV], F8)      # k^T   [dattn, lkv] (e4m3)
            v_sb = persist.tile([P, LC, D], BF)     # v     [lkv, dout]
            masks = mpool.tile([P, QT, LKV], F8)    # all mask rows (0/1 exact)

            # pair exchange buffers (AllGather within core pairs): each core
            # projects K/V for its half of the context; both halves come
            # back in group (= global) order.
            k_in = dram.tile([M, P, HKV], F8, name="k_in")
            k_out = dram.tile([2, M, P, HKV], F8, name="k_out")
            LH = LCH // 2  # 4 kv chunks per V-collective half
            v_in = [
                dram.tile([LH, P, D], BF, name=f"v_in{c}") for c in range(2)
            ]
            v_out = [
                dram.tile([2, LH, P, D], BF, name=f"v_out{c}") for c in range(2)
            ]
            RG = [[0, 1], [2, 3], [4, 5], [6, 7]]

            collective_in_body = reps == 1 or UNROLL_REPS
            if reps > 1:
                if UNROLL_REPS:
                    loop_ctx = None
                else:
                    loop_ctx = tc.For_i(0, reps, 1)
                    loop_ctx.__enter__()

            for _rep in range(reps if UNROLL_REPS else 1):
              # ---- phase 1: loads + transposes + Q/K/V projections ----
              with (
                  tc.tile_pool(name="w", bufs=1) as wp,
                  tc.tile_pool(name="xT", bufs=1) as xtp,
                  tc.tile_pool(name="cst", bufs=8) as cs,
                  tc.tile_pool(name="xst", bufs=8) as xs,
                  tc.tile_pool(name="kvst", bufs=4) as kvs,
              ):
                  wq_sb = wp.tile([P, DC, D], BF)
                  wk_sb = wp.tile([P, DC, D], BF)
                  wv_sb = wp.tile([P, DC, D], BF)

                  pT = xtp.tile([P, DC, LQ_LOC], BF)  # primary^T [din, lq]
                  cT_t = xtp.tile([P, DC, HKV], BF)   # ctx^T [din, own half]

                  def cT(dc, lo, width):
                      return cT_t[:, dc, lo : lo + width]

                  def pT_(dc, lo, width):
                      return pT[:, dc, lo : lo + width]

                  # ---- DMA issue order (SWDGE): K-proj critical path first
                  QW = D // 4
                  c_stage = []
                  for rb in range(NB):
                      st = cs.tile([P, D], BF, tag="cst", name=f"c{rb}")
                      nc.gpsimd.dma_start(st, ctx_d[bass.ts(rb, P), :])
                      c_stage.append(st)
                  for h in range(4):  # Wk in column quarters: m-pair h
                      nc.gpsimd.dma_start(
                          wk_sb[:, :, h * QW : (h + 1) * QW],
                          wk_d[:, h * QW : (h + 1) * QW].rearrange(
                              "(dc p) n -> p dc n", p=P
                          ),
                      )
                  nc.gpsimd.dma_start(
                      wv_sb, wv_d[:].rearrange("(dc p) n -> p dc n", p=P)
                  )
                  x_stage = []
                  for rb in range(NB):
                      st = xs.tile([P, D], BF, tag="xst", name=f"x{rb}")
                      nc.gpsimd.dma_start(st, x_d[bass.ts(rb, P), :])
                      x_stage.append(st)
                  nc.gpsimd.dma_start(
                      wq_sb, wq_d[:].rearrange("(dc p) n -> p dc n", p=P)
                  )
                  for qt in range(QT):  # masks for phase 2
                      nc.gpsimd.dma_start(masks[:, qt, :], mask_d[bass.ts(qt, P), :])

                  def t_block(stage, dst_T, rb):
                      for dc in range(DC):
                          tp = tps.tile([P, P], BF, tag="tp", name="tp")
                          nc.tensor.transpose(tp, stage[:, bass.ts(dc, P)], ident)
                          nc.vector.tensor_copy(dst_T[:, dc, bass.ts(rb, P)], tp)

                  def k_proj(l, m0, m1):
                      for mp in range(m0, m1, 2):
                          st = kvs.tile([P, 2, NT], F8, tag="kst", name="kst")
                          for mi in range(2):
                              m = mp + mi
                              ps = mmps.tile([P, NT], F32, tag="mm", name="ps")
                              for dc in range(DC):
                                  nc.tensor.matmul(
                                      ps,
                                      wk_sb[:, dc, bass.ts(m, P)],
                                      cT(dc, l * NT, NT),
                                      start=(dc == 0), stop=(dc == DC - 1),
                                  )
                              nc.scalar.activation(
                                  st[:, mi, :], ps, AF.Identity,
                                  bias=bk_sb[:, m : m + 1],
                              )
                          nc.sync.dma_start(
                              k_in[mp : mp + 2, :, l * NT : (l + 1) * NT]
                              .rearrange("mi p n -> p mi n"),
                              st[:],
                          )

                  for rb in range(NB):
                      t_block(c_stage[rb], cT_t, rb)
                  k_proj(0, 0, 8)
                  for n in range(D // NT):  # bv broadcast (not startup-critical)
                      ps = mmps.tile([P, NT], F32, tag="mm", name="ps")
                      nc.tensor.matmul(
                          ps, ones_row, bv_row[:, bass.ts(n, NT)],
                          start=True, stop=True,
                      )
                      nc.scalar.activation(bv_bcast[:, bass.ts(n, NT)], ps, AF.Copy)
                  k_proj(1, 0, 8)
                  if collective_in_body:
                      nc.gpsimd.collective_compute(
                          "AllGather", ALU.bypass, replica_groups=RG,
                          ins=[k_in[:]], outs=[k_out[:]],
                      )
                  else:  # timing stub: same bytes moved, no cross-core sync
                      nc.sync.dma_start(k_out[0], k_in[:])
                      nc.sync.dma_start(k_out[1], k_in[:])
                  # K gathers on the ACT ring (SP ring carries the V chain);
                  # queued ahead of phase-1 ACT evictions they must not block:
                  # released well before Q-proj needs ACT.
                  for r in range(2):
                      nc.scalar.dma_start(
                          kT[:, :, r * HKV : (r + 1) * HKV],
                          k_out[r].rearrange("m p h -> p m h"),
                      )

                  # V own half (natural layout; bias deferred), exchanged in
                  # two chunks so the collective+gather pipeline overlaps
                  # the remaining projections.
                  def v_proj_chunk(c):
                      for lh in range(LH):
                          lc = c * LH + lh
                          st = kvs.tile([P, 2, NT], BF, tag="vst", name="vst")
                          pss = [
                              mmps.tile([P, NT], F32, tag="mm", name="ps")
                              for _n in range(D // NT)
                          ]
                          for dc in range(DC):
                              for n in range(D // NT):
                                  nc.tensor.matmul(
                                      pss[n],
                                      cT(dc, lc * P, P),
                                      wv_sb[:, dc, bass.ts(n, NT)],
                                      start=(dc == 0), stop=(dc == DC - 1),
                                  )
                          for n in range(D // NT):
                              nc.vector.tensor_copy(st[:, n, :], pss[n])
                          nc.sync.dma_start(
                              v_in[c][lh], st[:].rearrange("p n nt -> p (n nt)")
                          )
                      if collective_in_body:
                          nc.gpsimd.collective_compute(
                              "AllGather", ALU.bypass, replica_groups=RG,
                              ins=[v_in[c][:]], outs=[v_out[c][:]],
                          )
                      else:  # timing stub
                          nc.sync.dma_start(v_out[c][0], v_in[c][:])
                          nc.sync.dma_start(v_out[c][1], v_in[c][:])
                      for r in range(2):
                          nc.sync.dma_start(
                              v_sb[:, r * LCH + c * LH : r * LCH + (c + 1) * LH, :],
                              v_out[c][r].rearrange("c p d -> p c d"),
                          )

                  v_proj_chunk(0)
                  v_proj_chunk(1)

                  # Q^T projection (evict fp8; DVE/ACT alternate)
                  for l in range(LQ_LOC // NT):
                      for rb in range(l * 4, (l + 1) * 4):
                          t_block(x_stage[rb], pT, rb)
                      for m in range(M):
                          ps = mmps.tile([P, NT], F32, tag="mm", name="ps")
                          for dc in range(DC):
                              nc.tensor.matmul(
                                  ps,
                                  wq_sb[:, dc, bass.ts(m, P)],
                                  pT_(dc, l * NT, NT),
                                  start=(dc == 0), stop=(dc == DC - 1),
                              )
                          if m % 2 == 0:  # alternate engines: halves the
                              nc.vector.tensor_scalar_add(  # eviction backlog
                                  qT[:, m, bass.ts(l, NT)], ps,
                                  bq_sb[:, m : m + 1],
                              )
                          else:
                              nc.scalar.activation(
                                  qT[:, m, bass.ts(l, NT)], ps, AF.Identity,
                                  bias=bq_sb[:, m : m + 1],
                              )

              # ---- phase 2: attention ----
              if PHASE1_ONLY:
                  continue
              with (
                  tc.tile_pool(name="epool", bufs=4) as epool,
                  tc.tile_pool(name="ptpool", bufs=4) as ptpool,
                  tc.tile_pool(name="rpool", bufs=8) as rpool,
                  tc.tile_pool(name="opool", bufs=3) as opool,
              ):
                  def s_phase(qt):
                      """S matmuls + mask fold + exp + XBAR P-transpose."""
                      e_sb = epool.tile([P, LKV], BF, tag="e", name="e_sb")
                      rs = rpool.tile([P, LT], F32, tag="rs", name="rs")
                      pt_sb = ptpool.tile([P, LC, P], BF, tag="pt", name="pt_sb")
                      # qT m-pair stays stationary across all 4 lt tiles
                      pss = [
                          mmps.tile([P, NT], F32, tag="mm", name="ps")
                          for _lt in range(LT)
                      ]
                      for m in range(0, M, 2):
                          for lt in range(LT):
                              nc.tensor.matmul(
                                  pss[lt],
                                  qT[:, m : m + 2, bass.ts(qt, P)],
                                  kT[:, m : m + 2, bass.ts(lt, NT)],
                                  start=(m == 0), stop=(m == M - 2),
                                  perf_mode=DR,
                              )
                      for lt in range(LT):
                          # S += -960 * mask (=> exp((S-960m)/32) = P * e^-30m)
                          nc.vector.scalar_tensor_tensor(
                              pss[lt], masks[:, qt, bass.ts(lt, NT)], -960.0,
                              pss[lt], op0=ALU.mult, op1=ALU.add,
                          )
                          nc.scalar.activation(
                              e_sb[:, bass.ts(lt, NT)], pss[lt], AF.Exp,
                              scale=1.0 / 32.0,
                              accum_out=rs[:, lt : lt + 1],
                          )
                          if lt % 2 == 1:  # XBAR transpose per finished half
                              h = lt // 2
                              nc.sync.dma_start_transpose(
                                  pt_sb[:, h * 8 : (h + 1) * 8, :],
                                  e_sb[:, h * 1024 : (h + 1) * 1024],
                              )
                      rsum = rpool.tile([P, 1], F32, tag="rsum", name="rsum")
                      recip = rpool.tile([P, 1], F32, tag="recip", name="recip")
                      nc.vector.reduce_sum(rsum, rs, axis=AX.X)
                      nc.vector.reciprocal(recip, rsum)
                      return pt_sb, recip

                  def av_phase(qt, pt_sb, recip):
                      """(P^T)^T @ V scaled by 1/rowsum, + bv, store."""
                      o_sb = opool.tile([P, D], F32, tag="o", name="o_sb")
                      av_ps = [
                          avps.tile([P, NT], F32, tag=f"av{n}", name=f"av{n}")
                          for n in range(D // NT)
                      ]
                      for half in range(2):  # pt[lc] stationary across n tiles
                          for lc in range(half * 8, half * 8 + 8):
                              for n in range(D // NT):
                                  nc.tensor.matmul(
                                      av_ps[n],
                                      pt_sb[:, lc, :],
                                      v_sb[:, lc, bass.ts(n, NT)],
                                      start=(lc == 0), stop=(lc == LC - 1),
                                  )
                      for n in range(D // NT):
                          nc.scalar.activation(
                              o_sb[:, bass.ts(n, NT)], av_ps[n],
                              AF.Identity, scale=recip[:, 0:1],
                          )
                          nc.vector.tensor_add(
                              o_sb[:, bass.ts(n, NT)],
                              o_sb[:, bass.ts(n, NT)],
                              bv_bcast[:, bass.ts(n, NT)],
                          )
                          nc.sync.dma_start(
                              out_d[bass.ts(qt, P), bass.ts(n, NT)],
                              o_sb[:, bass.ts(n, NT)],
                          )

                  # software pipeline: S/exp/transpose of tile qt overlaps
                  # AV of tile qt-1, hiding the exp+XBAR latency under AV.
                  prev = None
                  for qt in range(QT):
                      cur = s_phase(qt)
                      if prev is not None:
                          av_phase(qt - 1, *prev)
                      prev = cur
                  av_phase(QT - 1, *prev)

            if reps > 1 and loop_ctx is not None:
                loop_ctx.__exit__(None, None, None)

    nc.finalize()
    return nc


_NC_CACHE = None


def kernel(**inputs: np.ndarray) -> np.ndarray:
    global _NC_CACHE
    if _NC_CACHE is None:
        _NC_CACHE = build_nc()
    nc = _NC_CACHE

    primary = np.ascontiguousarray(np.asarray(inputs["primary"], dtype=np.float32))
    ctx = np.ascontiguousarray(
        np.asarray(inputs["context_sequence"], dtype=np.float32)
    )
    mask = np.ascontiguousarray(np.asarray(inputs["mask"], dtype=np.float32))
    shared = {
        k: np.ascontiguousarray(np.asarray(inputs[k], dtype=np.float32))
        for k in ("Wq", "bq", "Wk", "bk", "Wv", "bv")
    }

    H = LQ // 2  # 1024
    in_maps = []
    for c in range(8):
        b, h = c // 2, c % 2
        in_maps.append(
            {
                "primary": primary[b, h * H : (h + 1) * H, :],
                "context_sequence": np.ascontiguousarray(ctx[b, h * H : (h + 1) * H]),
                "mask": mask[b, h * H : (h + 1) * H, :],
                **shared,
            }
        )

    res = bass_utils.run_bass_kernel_spmd(nc, in_maps, core_ids=list(range(8)))

    out = np.empty((B, LQ, D), dtype=np.float32)
    for c in range(8):
        b, h = c // 2, c % 2
        out[b, h * H : (h + 1) * H, :] = res.results[c]["out"]
    return out


if __name__ == "__main__":
    rng = np.random.default_rng(0)
    ins = {
        "primary": rng.standard_normal((B, LQ, D), dtype=np.float32),
        "context_sequence": rng.standard_normal((B, LKV, D), dtype=np.float32),
        "mask": rng.integers(0, 2, (B, LQ, LKV)).astype(np.float32),
        "Wq": rng.uniform(-1 / 32, 1 / 32, (D, D)).astype(np.float32),
        "bq": rng.uniform(-1 / 32, 1 / 32, (D,)).astype(np.float32),
        "Wk": rng.uniform(-1 / 32, 1 / 32, (D, D)).astype(np.float32),
        "bk": rng.uniform(-1 / 32, 1 / 32, (D,)).astype(np.float32),
        "Wv": rng.uniform(-1 / 32, 1 / 32, (D, D)).astype(np.float32),
        "bv": rng.uniform(-1 / 32, 1 / 32, (D,)).astype(np.float32),
    }
    out = kernel(**ins)
    print("out", out.shape, out.dtype, float(np.abs(out).mean()))


# revision 47
# speedup vs baseline: 1.0939x; 1.0939x over previous
"""Fused attention kernel for Trainium2, SPMD over 8 NeuronCores.

Problem: nn_Attention_2808908611625
  q = primary @ Wq + bq;  k = ctx @ Wk + bk;  v = ctx @ Wv + bv
  out = softmax(q k^T / sqrt(1024) - 1e9 * mask) @ v

Sharding: core c handles batch b = c//2, query-row half h = c%2
  (1024 query rows per core, full K/V context of its batch, K/V projection
  split across the core pair and exchanged with a pair AllGather).

Per-core pipeline:
  1. SWDGE cast-DMA fp32->bf16 loads (W first: K-proj critical path), PE
     128x128 transposes put the contraction dim on SBUF partitions.
  2. Q/K/V projections on PE (bf16, fp32 PSUM); q/k evicted as fp8 e4m3
     (bias folded); K exchanged through DRAM in fp8; bv added at the end
     (softmax rows sum to 1 => attn @ (1 bv^T) = bv).
  3. S = qT.T @ kT with fp8 DoubleRow matmuls ([128 x 512] PSUM tiles);
     mask folded with one DVE scalar_tensor_tensor (S += -960 * mask);
     P = exp(S/32) via ACT with accum_out row-sums. No max-subtraction:
     |S/32| <= ~4 unmasked, masked entries become exp(-30) ~ 1e-13.
  4. P tiles transposed by XBAR DMA (SBUF->SBUF, one per query tile),
     PV matmul in bf16, evict with per-partition 1/rowsum scale, add
     broadcast bv, one fp32 DMA out per query tile.
"""

import numpy as np

import concourse.bass as bass
import concourse.mybir as mybir
import concourse.tile as tile
from concourse import bacc, bass_utils
from concourse.masks import make_identity

BF = mybir.dt.bfloat16
F8 = mybir.dt.float8e4
F32 = mybir.dt.float32
AF = mybir.ActivationFunctionType
ALU = mybir.AluOpType
AX = mybir.AxisListType
DR = mybir.MatmulPerfMode.DoubleRow

B, LQ, LKV, D = 4, 2048, 2048, 1024
P = 128
LQ_LOC = (B * LQ) // 8  # 1024 query rows per core
DC = D // P             # 8 contraction chunks
M = D // P              # 8 output-dim chunks
QT = LQ_LOC // P        # 8 query tiles per core
NT = 512                # moving free dim / psum tile width
LT = LKV // NT          # 4 kv column tiles for S
LC = LKV // P           # 16 kv chunks for PV
HKV = LKV // 2          # per-core K/V rows (pair-sharded)
LTH = HKV // NT         # 2 own kv column tiles
LCH = HKV // P          # 8 own kv chunks
NB = LQ_LOC // P        # 8 row blocks per input tensor

UNROLL_REPS = False
PHASE1_ONLY = False  # timing probe: stop after projections+exchange


def build_nc(reps: int = 1):
    nc = bacc.Bacc("TRN2", num_swdge_queues=4, num_devices=8)

    x_d = nc.dram_tensor("primary", (LQ_LOC, D), F32, kind="ExternalInput")
    ctx_d = nc.dram_tensor("context_sequence", (LKV // 2, D), F32, kind="ExternalInput")
    mask_d = nc.dram_tensor("mask", (LQ_LOC, LKV), F32, kind="ExternalInput")
    wq_d = nc.dram_tensor("Wq", (D, D), F32, kind="ExternalInput")
    bq_d = nc.dram_tensor("bq", (D,), F32, kind="ExternalInput")
    wk_d = nc.dram_tensor("Wk", (D, D), F32, kind="ExternalInput")
    bk_d = nc.dram_tensor("bk", (D,), F32, kind="ExternalInput")
    wv_d = nc.dram_tensor("Wv", (D, D), F32, kind="ExternalInput")
    bv_d = nc.dram_tensor("bv", (D,), F32, kind="ExternalInput")
    out_d = nc.dram_tensor("out", (LQ_LOC, D), F32, kind="ExternalOutput")

    with tile.TileContext(nc) as tc:
        with (
            tc.tile_pool(name="const", bufs=1) as const,
            tc.tile_pool(name="persist", bufs=1) as persist,
            tc.tile_pool(name="mpool", bufs=1) as mpool,
            tc.tile_pool(name="dram", bufs=1, space="DRAM") as dram,
            tc.tile_pool(name="mmps", bufs=4, space="PSUM") as mmps,
            tc.tile_pool(name="tps", bufs=2, space="PSUM") as tps,
            tc.tile_pool(name="avps", bufs=1, space="PSUM") as avps,
        ):
            ident = const.tile([P, P], BF)
            make_identity(nc, ident)

            # biases: b*_sb[p, m] = b[m*128 + p]
            bq_sb = const.tile([P, M], F32)
            bk_sb = const.tile([P, M], F32)
            with nc.allow_non_contiguous_dma(reason="tiny bias vectors"):
                nc.sync.dma_start(bq_sb, bq_d[:].rearrange("(m p) -> p m", p=P))
                nc.sync.dma_start(bk_sb, bk_d[:].rearrange("(m p) -> p m", p=P))

            # bv broadcast to all partitions: ones[1,128].T @ bv[1, D]
            bv_row = const.tile([1, D], BF)
            ones_row = const.tile([1, P], BF)
            nc.vector.memset(ones_row, 1.0)
            bv_bcast = const.tile([P, D], F32)

            qT = persist.tile([P, M, LQ_LOC], F8)   # q^T   [dattn, lq] (e4m3)
            kT = persist.tile([P, M, LKV], F8)      # k^T   [dattn, lkv] (e4m3)
            v_sb = persist.tile([P, LC, D], BF)     # v     [lkv, dout]
            masks = mpool.tile([P, QT, LKV], F8)    # all mask rows (0/1 exact)

            # pair exchange buffers (AllGather within core pairs): each core
            # projects K/V for its half of the context; both halves come
            # back in group (= global) order.
            k_in = dram.tile([M, P, HKV], F8, name="k_in")
            k_out = dram.tile([2, M, P, HKV], F8, name="k_out")
            LH = LCH // 2  # 4 kv chunks per V-collective half
            v_in = [
                dram.tile([LH, P, D], BF, name=f"v_in{c}") for c in range(2)
            ]
            v_out = [
                dram.tile([2, LH, P, D], BF, name=f"v_out{c}") for c in range(2)
            ]
            RG = [[0, 1], [2, 3], [4, 5], [6, 7]]

            collective_in_body = reps == 1 or UNROLL_REPS
            if reps > 1:
                if UNROLL_REPS:
                    loop_ctx = None
                else:
                    loop_ctx = tc.For_i(0, reps, 1)
                    loop_ctx.__enter__()

            for _rep in range(reps if UNROLL_REPS else 1):
              # ---- phase 1: loads + transposes + Q/K/V projections ----
              with (
                  tc.tile_pool(name="w", bufs=1) as wp,
                  tc.tile_pool(name="xT", bufs=1) as xtp,
                  tc.tile_pool(name="cst", bufs=8) as cs,
                  tc.tile_pool(name="xst", bufs=8) as xs,
                  tc.tile_pool(name="kvst", bufs=4) as kvs,
              ):
                  wq_sb = wp.tile([P, DC, D], BF)
                  wk_sb = wp.tile([P, DC, D], BF)
                  wv_sb = wp.tile([P, DC, D], BF)

                  pT = xtp.tile([P, DC, LQ_LOC], BF)  # primary^T [din, lq]
                  cT_t = xtp.tile([P, DC, HKV], BF)   # ctx^T [din, own half]

                  def cT(dc, lo, width):
                      return cT_t[:, dc, lo : lo + width]

                  def pT_(dc, lo, width):
                      return pT[:, dc, lo : lo + width]

                  # ---- DMA issue order (SWDGE): K-proj critical path first
                  QW = D // 4

                  def wk_quarter(h):
                      nc.gpsimd.dma_start(
                          wk_sb[:, :, h * QW : (h + 1) * QW],
                          wk_d[:, h * QW : (h + 1) * QW].rearrange(
                              "(dc p) n -> p dc n", p=P
                          ),
                      )

                  c_stage = []
                  for rb in range(NB):  # ctx b0-3, wk q0-1, ctx b4-7, wk q2-3
                      st = cs.tile([P, D], BF, tag="cst", name=f"c{rb}")
                      nc.gpsimd.dma_start(st, ctx_d[bass.ts(rb, P), :])
                      c_stage.append(st)
                      if rb == 3:
                          wk_quarter(0)
                          wk_quarter(1)
                  wk_quarter(2)
                  wk_quarter(3)
                  nc.gpsimd.dma_start(
                      wv_sb, wv_d[:].rearrange("(dc p) n -> p dc n", p=P)
                  )
                  nc.gpsimd.dma_start(
                      bv_row, bv_d[:].rearrange("(one n) -> one n", one=1)
                  )
                  x_stage = []
                  for rb in range(NB):
                      st = xs.tile([P, D], BF, tag="xst", name=f"x{rb}")
                      nc.gpsimd.dma_start(st, x_d[bass.ts(rb, P), :])
                      x_stage.append(st)
                  nc.gpsimd.dma_start(
                      wq_sb, wq_d[:].rearrange("(dc p) n -> p dc n", p=P)
                  )
                  for qt in range(QT):  # masks for phase 2
                      nc.gpsimd.dma_start(masks[:, qt, :], mask_d[bass.ts(qt, P), :])

                  def t_block(stage, dst_T, rb):
                      for dc in range(DC):
                          tp = tps.tile([P, P], BF, tag="tp", name="tp")
                          nc.tensor.transpose(tp, stage[:, bass.ts(dc, P)], ident)
                          nc.vector.tensor_copy(dst_T[:, dc, bass.ts(rb, P)], tp)

                  def k_proj(l, m0, m1):
                      for mp in range(m0, m1, 2):
                          st = kvs.tile([P, 2, NT], F8, tag="kst", name="kst")
                          for mi in range(2):
                              m = mp + mi
                              ps = mmps.tile([P, NT], F32, tag="mm", name="ps")
                              for dc in range(DC):
                                  nc.tensor.matmul(
                                      ps,
                                      wk_sb[:, dc, bass.ts(m, P)],
                                      cT(dc, l * NT, NT),
                                      start=(dc == 0), stop=(dc == DC - 1),
                                  )
                              nc.scalar.activation(
                                  st[:, mi, :], ps, AF.Identity,
                                  bias=bk_sb[:, m : m + 1],
                              )
                          nc.sync.dma_start(
                              k_in[mp : mp + 2, :, l * NT : (l + 1) * NT]
                              .rearrange("mi p n -> p mi n"),
                              st[:],
                          )

                  for rb in range(4):
                      t_block(c_stage[rb], cT_t, rb)
                  k_proj(0, 0, 4)
                  for rb in range(4, NB):
                      t_block(c_stage[rb], cT_t, rb)
                  k_proj(0, 4, 8)
                  for n in range(D // NT):  # bv broadcast (not startup-critical)
                      ps = mmps.tile([P, NT], F32, tag="mm", name="ps")
                      nc.tensor.matmul(
                          ps, ones_row, bv_row[:, bass.ts(n, NT)],
                          start=True, stop=True,
                      )
                      nc.scalar.activation(bv_bcast[:, bass.ts(n, NT)], ps, AF.Copy)
                  k_proj(1, 0, 8)
                  if collective_in_body:
                      nc.gpsimd.collective_compute(
                          "AllGather", ALU.bypass, replica_groups=RG,
                          ins=[k_in[:]], outs=[k_out[:]],
                      )
                  else:  # timing stub: same bytes moved, no cross-core sync
                      nc.sync.dma_start(k_out[0], k_in[:])
                      nc.sync.dma_start(k_out[1], k_in[:])
                  # K gathers on the ACT ring (SP ring carries the V chain);
                  # queued ahead of phase-1 ACT evictions they must not block:
                  # released well before Q-proj needs ACT.
                  for r in range(2):
                      nc.scalar.dma_start(
                          kT[:, :, r * HKV : (r + 1) * HKV],
                          k_out[r].rearrange("m p h -> p m h"),
                      )

                  # V own half (natural layout; bias deferred), exchanged in
                  # two chunks so the collective+gather pipeline overlaps
                  # the remaining projections.
                  def v_proj_chunk(c):
                      for lh in range(LH):
                          lc = c * LH + lh
                          st = kvs.tile([P, 2, NT], BF, tag="vst", name="vst")
                          pss = [
                              mmps.tile([P, NT], F32, tag="mm", name="ps")
                              for _n in range(D // NT)
                          ]
                          for dc in range(DC):
                              for n in range(D // NT):
                                  nc.tensor.matmul(
                                      pss[n],
                                      cT(dc, lc * P, P),
                                      wv_sb[:, dc, bass.ts(n, NT)],
                                      start=(dc == 0), stop=(dc == DC - 1),
                                  )
                          for n in range(D // NT):
                              nc.vector.tensor_copy(st[:, n, :], pss[n])
                          nc.sync.dma_start(
                              v_in[c][lh], st[:].rearrange("p n nt -> p (n nt)")
                          )
                      if collective_in_body:
                          nc.gpsimd.collective_compute(
                              "AllGather", ALU.bypass, replica_groups=RG,
                              ins=[v_in[c][:]], outs=[v_out[c][:]],
                          )
                      else:  # timing stub
                          nc.sync.dma_start(v_out[c][0], v_in[c][:])
                          nc.sync.dma_start(v_out[c][1], v_in[c][:])
                      for r in range(2):
                          nc.sync.dma_start(
                              v_sb[:, r * LCH + c * LH : r * LCH + (c + 1) * LH, :],
                              v_out[c][r].rearrange("c p d -> p c d"),
                          )

                  v_proj_chunk(0)
                  v_proj_chunk(1)

                  # Q^T projection (evict fp8; DVE/ACT alternate)
                  for l in range(LQ_LOC // NT):
                      for rb in range(l * 4, (l + 1) * 4):
                          t_block(x_stage[rb], pT, rb)
                      for m in range(M):
                          ps = mmps.tile([P, NT], F32, tag="mm", name="ps")
                          for dc in range(DC):
                              nc.tensor.matmul(
                                  ps,
                                  wq_sb[:, dc, bass.ts(m, P)],
                                  pT_(dc, l * NT, NT),
                                  start=(dc == 0), stop=(dc == DC - 1),
                              )
                          if m % 2 == 0:  # alternate engines: halves the
                              nc.vector.tensor_scalar_add(  # eviction backlog
                                  qT[:, m, bass.ts(l, NT)], ps,
                                  bq_sb[:, m : m + 1],
                              )
                          else:
                              nc.scalar.activation(
                                  qT[:, m, bass.ts(l, NT)], ps, AF.Identity,
                                  bias=bq_sb[:, m : m + 1],
                              )

              # ---- phase 2: attention ----
              if PHASE1_ONLY:
                  continue
              with (
                  tc.tile_pool(name="epool", bufs=4) as epool,
                  tc.tile_pool(name="ptpool", bufs=4) as ptpool,
                  tc.tile_pool(name="rpool", bufs=8) as rpool,
                  tc.tile_pool(name="opool", bufs=3) as opool,
              ):
                  def s_phase(qt):
                      """S matmuls + mask fold + exp + XBAR P-transpose."""
                      e_sb = epool.tile([P, LKV], BF, tag="e", name="e_sb")
                      rs = rpool.tile([P, LT], F32, tag="rs", name="rs")
                      pt_sb = ptpool.tile([P, LC, P], BF, tag="pt", name="pt_sb")
                      # qT m-pair stays stationary across all 4 lt tiles
                      pss = [
                          mmps.tile([P, NT], F32, tag="mm", name="ps")
                          for _lt in range(LT)
                      ]
                      for m in range(0, M, 2):
                          for lt in range(LT):
                              nc.tensor.matmul(
                                  pss[lt],
                                  qT[:, m : m + 2, bass.ts(qt, P)],
                                  kT[:, m : m + 2, bass.ts(lt, NT)],
                                  start=(m == 0), stop=(m == M - 2),
                                  perf_mode=DR,
                              )
                      for lt in range(LT):
                          # S += -960 * mask (=> exp((S-960m)/32) = P * e^-30m)
                          nc.vector.scalar_tensor_tensor(
                              pss[lt], masks[:, qt, bass.ts(lt, NT)], -960.0,
                              pss[lt], op0=ALU.mult, op1=ALU.add,
                          )
                          nc.scalar.activation(
                              e_sb[:, bass.ts(lt, NT)], pss[lt], AF.Exp,
                              scale=1.0 / 32.0,
                              accum_out=rs[:, lt : lt + 1],
                          )
                          if lt % 2 == 1:  # XBAR transpose per finished half
                              h = lt // 2
                              nc.sync.dma_start_transpose(
                                  pt_sb[:, h * 8 : (h + 1) * 8, :],
                                  e_sb[:, h * 1024 : (h + 1) * 1024],
                              )
                      rsum = rpool.tile([P, 1], F32, tag="rsum", name="rsum")
                      recip = rpool.tile([P, 1], F32, tag="recip", name="recip")
                      nc.vector.reduce_sum(rsum, rs, axis=AX.X)
                      nc.vector.reciprocal(recip, rsum)
                      return pt_sb, recip

                  def av_phase(qt, pt_sb, recip):
                      """(P^T)^T @ V scaled by 1/rowsum, + bv, store."""
                      o_sb = opool.tile([P, D], F32, tag="o", name="o_sb")
                      av_ps = [
                          avps.tile([P, NT], F32, tag=f"av{n}", name=f"av{n}")
                          for n in range(D // NT)
                      ]
                      for half in range(2):  # pt[lc] stationary across n tiles
                          for lc in range(half * 8, half * 8 + 8):
                              for n in range(D // NT):
                                  nc.tensor.matmul(
                                      av_ps[n],
                                      pt_sb[:, lc, :],
                                      v_sb[:, lc, bass.ts(n, NT)],
                                      start=(lc == 0), stop=(lc == LC - 1),
                                  )
                      for n in range(D // NT):
                          nc.scalar.activation(
                              o_sb[:, bass.ts(n, NT)], av_ps[n],
                              AF.Identity, scale=recip[:, 0:1],
                          )
                          nc.vector.tensor_add(
                              o_sb[:, bass.ts(n, NT)],
                              o_sb[:, bass.ts(n, NT)],
                              bv_bcast[:, bass.ts(n, NT)],
                          )
                          nc.sync.dma_start(
                              out_d[bass.ts(qt, P), bass.ts(n, NT)],
                              o_sb[:, bass.ts(n, NT)],
                          )

                  # software pipeline: S/exp/transpose of tile qt overlaps
                  # AV of tile qt-1, hiding the exp+XBAR latency under AV.
                  prev = None
                  for qt in range(QT):
                      cur = s_phase(qt)
                      if prev is not None:
                          av_phase(qt - 1, *prev)
                      prev = cur
                  av_phase(QT - 1, *prev)

            if reps > 1 and loop_ctx is not None:
                loop_ctx.__exit__(None, None, None)

    nc.finalize()
    return nc


_NC_CACHE = None


def kernel(**inputs: np.ndarray) -> np.ndarray:
    global _NC_CACHE
    if _NC_CACHE is None:
        _NC_CACHE = build_nc()
    nc = _NC_CACHE

    primary = np.ascontiguousarray(np.asarray(inputs["primary"], dtype=np.float32))
    ctx = np.ascontiguousarray(
        np.asarray(inputs["context_sequence"], dtype=np.float32)
    )
    mask = np.ascontiguousarray(np.asarray(inputs["mask"], dtype=np.float32))
    shared = {
        k: np.ascontiguousarray(np.asarray(inputs[k], dtype=np.float32))
        for k in ("Wq", "bq", "Wk", "bk", "Wv", "bv")
    }

    H = LQ // 2  # 1024
    in_maps = []
    for c in range(8):
        b, h = c // 2, c % 2
        in_maps.append(
            {
                "primary": primary[b, h * H : (h + 1) * H, :],
                "context_sequence": np.ascontiguousarray(ctx[b, h * H : (h + 1) * H]),
                "mask": mask[b, h * H : (h + 1) * H, :],
                **shared,
            }
        )

    res = bass_utils.run_bass_kernel_spmd(nc, in_maps, core_ids=list(range(8)))

    out = np.empty((B, LQ, D), dtype=np.float32)
    for c in range(8):
        b, h = c // 2, c % 2
        out[b, h * H : (h + 1) * H, :] = res.results[c]["out"]
    return out


if __name__ == "__main__":
    rng = np.random.default_rng(0)
    ins = {
        "primary": rng.standard_normal((B, LQ, D), dtype=np.float32),
        "context_sequence": rng.standard_normal((B, LKV, D), dtype=np.float32),
        "mask": rng.integers(0, 2, (B, LQ, LKV)).astype(np.float32),
        "Wq": rng.uniform(-1 / 32, 1 / 32, (D, D)).astype(np.float32),
        "bq": rng.uniform(-1 / 32, 1 / 32, (D,)).astype(np.float32),
        "Wk": rng.uniform(-1 / 32, 1 / 32, (D, D)).astype(np.float32),
        "bk": rng.uniform(-1 / 32, 1 / 32, (D,)).astype(np.float32),
        "Wv": rng.uniform(-1 / 32, 1 / 32, (D, D)).astype(np.float32),
        "bv": rng.uniform(-1 / 32, 1 / 32, (D,)).astype(np.float32),
    }
    out = kernel(**ins)
    print("out", out.shape, out.dtype, float(np.abs(out).mean()))
